# revision 8
# baseline (speedup 1.0000x reference)
"""LoRA-attention TRN2 kernel: head-tensor-parallel over 8 NeuronCores.

Problem (hardcoded): x [4, 2048, 2048] f32, causal mask, H=16 heads, HD=128,
LoRA rank 16 on all four projections.

Strategy:
  - Host folds LoRA into the weights:  W_eff^T = W^T + A^T @ B^T  (exact),
    then casts x and all weights to bf16 (PE streams bf16 at the same
    1 cycle/row as f32r but DMA/SBUF/DVE traffic halves; PSUM stays fp32).
  - Tensor-parallel over heads: core c owns heads {2c, 2c+1} = feature slice
    [c*256, (c+1)*256).  Each core computes Q^T/K^T (feature-major) and V
    (token-major) for its slice, causal flash-style attention per (batch,
    head) with logits kept transposed ([key, query]) so the attention
    contraction stays on the partition dim, then a partial output projection
    over its 256 features.  Host sums the 8 bf16 partial outputs in fp32.
  - Softmax: no max-subtraction (logits are O(1); exp of masked -inf never
    occurs because masked tiles are either skipped or zeroed post-exp).
    Column sums via ones-vector matmul on the PE; normalization per query
    tile (first tile of each pair mid-block) via DVE reciprocal + Pool
    broadcast + DVE multiply.
  - Diagonal narrowing: for key chunk jt = 4*it + d only queries f >= 128*d
    of tile it attend, so logit/AV/sum matmuls cover [128*d, 512) and the
    partial mask shrinks to a fixed [P,128] slice — exact, no extra error.
  - Scheduling: weights load in (wq,wk,wv)-lockstep kc chunks on the ACT
    queue while x tiles stream on the sync queue (startup ~4us, was 30us);
    QK and V projection passes alternate PSUM slot pairs; attention logits
    are software-pipelined one work-unit ahead of the AV/sum matmuls with
    one union-range exp per unit (the causal tail pairs two key chunks per
    unit, halving ACT instructions where exp is the phase bottleneck);
    PSUM->SBUF copies are whole-tile strided (one ACT + one DVE instr per
    PSUM tile in projections and output projection) with out-DMAs on the
    sync queue.
    TimelineSim: 662us vs 861us for the f32r baseline (HW 1054904ns).
"""

import math
import os
import sys

import numpy as np

sys.path.insert(0, "/opt/trn_rl_repo")

B, S, D, H, R = 4, 2048, 2048, 16, 16
HD = D // H              # 128
NCORES = 8
HPC = H // NCORES        # heads per core = 2
FPC = HPC * HD           # features per core = 256
T = B * S                # 8192 tokens
P = 128
SCALE = 1.0 / math.sqrt(HD)

_COMPILED = {}


def _build_nc(causal: bool, iters: int = 1):
    import concourse.mybir as mybir
    import concourse.tile as tile
    from concourse import bacc

    f32 = mybir.dt.float32
    f32r = mybir.dt.float32r
    bf16 = mybir.dt.bfloat16
    nc = bacc.Bacc("TRN2", target_bir_lowering=False, debug=False)

    xt = nc.dram_tensor("xt", [D, T], bf16, kind="ExternalInput")
    wq = nc.dram_tensor("wq", [D, FPC], bf16, kind="ExternalInput")
    wk = nc.dram_tensor("wk", [D, FPC], bf16, kind="ExternalInput")
    wv = nc.dram_tensor("wv", [D, FPC], bf16, kind="ExternalInput")
    wo = nc.dram_tensor("wo", [FPC, D], bf16, kind="ExternalInput")
    dm = nc.dram_tensor("dm", [4 * P, 512], bf16, kind="ExternalInput")
    out = nc.dram_tensor("out", [T, D], bf16, kind="ExternalOutput")

    KC = D // P            # 16 contraction chunks for projections
    NTG = S // 512         # 4 token groups per batch
    NIT = S // 512         # 4 query tiles per (b, h)
    NJT = S // P           # 16 key chunks per (b, h)

    with tile.TileContext(nc) as tc:
        with (
            nc.allow_low_precision(reason="f32r matmul pipeline; fp32 PSUM accum"),
            tc.tile_pool(name="consts", bufs=1) as consts,
            tc.tile_pool(name="xp", bufs=16) as xp,
            tc.tile_pool(name="qk", bufs=2) as qkp,
            tc.tile_pool(name="vp", bufs=2) as vpool,
            tc.tile_pool(name="ep", bufs=6) as ep,
            tc.tile_pool(name="ot", bufs=2) as otp_pool,
            tc.tile_pool(name="sm", bufs=4) as smp,
            tc.tile_pool(name="ob", bufs=6) as obp,
            tc.tile_pool(name="ps", bufs=4, space="PSUM") as ps,
        ):
            # ---- resident constants ----
            # Weights load on the DVE queue in (wq,wk,wv)-lockstep kc chunks
            # so the first projection matmuls unblock after ~1MB of traffic
            # instead of the full 8.4MB; x tiles stream on the sync queue in
            # parallel.  dm/wo aren't needed until attention/outproj: last.
            wq_sb = consts.tile([P, KC, FPC], bf16)
            wk_sb = consts.tile([P, KC, FPC], bf16)
            wv_sb = consts.tile([P, KC, FPC], bf16)
            wo_sb = consts.tile([P, HPC, D], bf16)
            for lo, hi in ((0, 2), (2, 4), (4, 8), (8, 16)):
                for wsb, wdr in ((wq_sb, wq), (wk_sb, wk), (wv_sb, wv)):
                    nc.scalar.dma_start(
                        out=wsb[:, lo:hi, :],
                        in_=wdr.ap()[lo * P:hi * P, :].rearrange(
                            "(c p) f -> p c f", p=P),
                    )
            dm_sb = consts.tile([P, 4, 512], bf16)
            nc.scalar.dma_start(out=dm_sb, in_=dm.ap().rearrange("(d p) i -> p d i", p=P))
            nc.scalar.dma_start(out=wo_sb, in_=wo.ap().rearrange("(h p) e -> p h e", p=P))
            ones_col_f = consts.tile([P, 1], f32)
            nc.any.memset(ones_col_f, 1.0)
            ones_col_r = consts.tile([P, 1], bf16)   # K=128, M=1 sums weight
            nc.vector.tensor_copy(ones_col_r, ones_col_f)

            def alloc_tiles(b):
                return dict(
                    qt=qkp.tile([P, HPC, S], bf16, tag="qt", name=f"qt{b}"),
                    kt=qkp.tile([P, HPC, S], bf16, tag="kt", name=f"kt{b}"),
                    v=vpool.tile([P, NJT, FPC], bf16, tag="v", name=f"v{b}"),
                    ot=otp_pool.tile([P, HPC, S], bf16, tag="ot",
                                     name=f"ot{b}"),
                )

            def body():
                # Software pipeline across batches: the attention phase of
                # batch b (PE ~50% busy due to exp latency + PSUM-release
                # waits) is interleaved at block granularity with the QKV
                # projection of batch b+1 and the output projection of
                # batch b-1 (both dense PE streams).  This keeps the PE
                # saturated and the HAM clock-gate warm through attention.
                tiles = [None] * B
                tiles[0] = alloc_tiles(0)
                for g in range(NTG):
                    _proj_group(0, tiles[0], g)
                for b in range(B):
                    fillers = []
                    pg = []
                    if b + 1 < B:
                        tiles[b + 1] = alloc_tiles(b + 1)
                        pg = [
                            (lambda g=g, b1=b + 1: _proj_group(b1, tiles[b1], g))
                            for g in range(NTG)
                        ]
                    op = []
                    if b - 1 >= 0:
                        def _op_chunk(b0, m4):
                            for mt in range(4 * m4, 4 * m4 + 4):
                                _outproj_mt(b0, tiles[b0], mt)
                        op = [
                            (lambda b0=b - 1, m4=m4: _op_chunk(b0, m4))
                            for m4 in range(4)
                        ]
                    # alternate proj / outproj fillers
                    for i in range(max(len(pg), len(op))):
                        if i < len(pg):
                            fillers.append(pg[i])
                        if i < len(op):
                            fillers.append(op[i])
                    blocks = [(h, ip) for h in range(HPC)
                              for ip in range(NIT // 2)]
                    nb = len(blocks)
                    for i, (h, ip) in enumerate(blocks):
                        _attn_block(b, tiles[b], h, ip)
                        lo = (i * len(fillers)) // nb
                        hi = ((i + 1) * len(fillers)) // nb
                        for f in fillers[lo:hi]:
                            f()
                # drain: output projection of the last batch
                for mt in range(S // P):
                    _outproj_mt(B - 1, tiles[B - 1], mt)

            # Two passes per token group over the same resident x tiles:
            # Q/K (2 PSUM slots) then V (2 slots).  Each pass's PSUM
            # copies overlap the other pass's matmuls, so token-group
            # transitions never stall on slot reuse.
            def _proj_group(b, t, g):
                    qt_sb, kt_sb, v_sb = t["qt"], t["kt"], t["v"]
                    toff = b * S + g * 512
                    xts = []
                    qp2 = ps.tile([P, 2, 512], f32, tag="ps", name=f"qp{b}_{g}")
                    kp2 = ps.tile([P, 2, 512], f32, tag="ps", name=f"kp{b}_{g}")
                    for kc2 in range(KC // 2):
                        xt_t = xp.tile([P, 2, 512], bf16, tag="xt")
                        nc.sync.dma_start(
                            out=xt_t,
                            in_=xt.ap()[
                                kc2 * 2 * P:(kc2 + 1) * 2 * P, toff:toff + 512
                            ].rearrange("(two p) t -> p two t", p=P),
                        )
                        xts.append(xt_t)
                        for j in range(2):
                            kc = 2 * kc2 + j
                            st = dict(start=(kc == 0), stop=(kc == KC - 1))
                            for m in range(HPC):
                                nc.tensor.matmul(
                                    qp2[:, m, :], wq_sb[:, kc, m * P:(m + 1) * P],
                                    xt_t[:, j, :], **st
                                )
                                nc.tensor.matmul(
                                    kp2[:, m, :], wk_sb[:, kc, m * P:(m + 1) * P],
                                    xt_t[:, j, :], **st
                                )
                    # Whole-tile strided copies: both head halves in one
                    # instruction per PSUM tile.
                    nc.scalar.copy(qt_sb[:, :, g * 512:(g + 1) * 512], qp2[:, :, :])
                    nc.vector.tensor_copy(
                        kt_sb[:, :, g * 512:(g + 1) * 512], kp2[:, :, :])

                    vp2a = ps.tile([P, 2, 512], f32, tag="ps", name=f"va{b}_{g}")
                    vp2b = ps.tile([P, 2, 512], f32, tag="ps", name=f"vb{b}_{g}")
                    def vp_slice(ts):
                        t = vp2a if ts < 2 else vp2b
                        return t[:, ts % 2, 0:FPC]
                    for kc2 in range(KC // 2):
                        xt_t = xts[kc2]
                        for j in range(2):
                            kc = 2 * kc2 + j
                            st = dict(start=(kc == 0), stop=(kc == KC - 1))
                            for ts in range(4):
                                nc.tensor.matmul(
                                    vp_slice(ts), xt_t[:, j, ts * P:(ts + 1) * P],
                                    wv_sb[:, kc, :], **st
                                )
                    nc.scalar.copy(v_sb[:, g * 4:g * 4 + 2, :],
                                   vp2a[:, 0:2, 0:FPC])
                    nc.vector.tensor_copy(v_sb[:, g * 4 + 2:g * 4 + 4, :],
                                          vp2b[:, 0:2, 0:FPC])

            # ---------- attention block (jt-outer, query-tile pairs;
            # K/V/ones weight loads amortized, exp over tile pairs) ----
            def _attn_block(b, t, h, ip):
                        qt_sb, kt_sb = t["qt"], t["kt"]
                        v_sb, ot_sb = t["v"], t["ot"]
                        pits = (2 * ip, 2 * ip + 1)
                        otp2 = ps.tile([P, 2, 512], f32, tag="ps",
                                       name=f"otp{b}_{h}_{ip}")
                        sp2 = ps.tile([1, 2, 512], f32, tag="ps",
                                      name=f"sp{b}_{h}_{ip}")
                        jmax = (4 * pits[-1] + 4) if causal else NJT

                        def its_of(jt):
                            return [it for it in pits
                                    if (not causal) or jt <= 4 * it + 3]

                        def qlo_of(it, jt):
                            # Diagonal narrowing: key chunk jt = 4*it + d is
                            # only attended by queries f >= 128*d of tile it.
                            if causal and jt // 4 == it:
                                return P * (jt - 4 * it)
                            return 0

                        # Work units: head region processes one key chunk for
                        # both query tiles per unit; the causal tail (only the
                        # pair's second tile active, all-diagonal) processes
                        # TWO key chunks per unit sharing one lp tile, so one
                        # exp covers both — halves ACT instructions where ACT
                        # is the phase bottleneck.
                        if causal:
                            hmax = 4 * pits[0] + 4
                            units = ([("h", jt) for jt in range(hmax)]
                                     + [("t", j0) for j0 in range(hmax, jmax, 2)])
                        else:
                            units = [("h", jt) for jt in range(jmax)]

                        # Software pipeline: logits for unit u+1 are issued
                        # before the AV/sum matmuls of unit u, so PE streams
                        # through logits while ACT computes exp(u).
                        lp_live = {}

                        def issue_logits(ui):
                            kind, j0 = units[ui]
                            lp2 = ps.tile([P, 2, 512], f32, tag="ps",
                                          name=f"lp{b}_{h}_{ip}_{ui}")
                            if kind == "h":
                                for it in its_of(j0):
                                    q0 = qlo_of(it, j0)
                                    nc.tensor.matmul(
                                        lp2[:, it % 2, q0:512],
                                        kt_sb[:, h, j0 * P:(j0 + 1) * P],
                                        qt_sb[:, h, it * 512 + q0:(it + 1) * 512],
                                        start=True, stop=True,
                                    )
                            else:
                                it = pits[1]
                                for k in range(2):
                                    jt = j0 + k
                                    q0 = qlo_of(it, jt)
                                    nc.tensor.matmul(
                                        lp2[:, k, q0:512],
                                        kt_sb[:, h, jt * P:(jt + 1) * P],
                                        qt_sb[:, h, it * 512 + q0:(it + 1) * 512],
                                        start=True, stop=True,
                                    )
                            lp_live[ui] = lp2

                        def _normalize(it):
                            # recip -> Pool broadcast -> DVE mul for one
                            # query tile.  For the first tile of the pair
                            # this runs mid-block (its sums stop early), so
                            # only the second tile's chain sits on the
                            # block-end critical path.
                            rinv = smp.tile([1, 512], f32, tag="rinv")
                            # approx recip (18 bits, ~5x faster than
                            # reciprocal): keeps the block-end PSUM-release
                            # chain off the PE critical path.
                            nc.vector.reciprocal_approx_fast(
                                rinv, sp2[0:1, it % 2, :])
                            rbs = smp.tile([P, 512], f32, tag="rbs")
                            nc.gpsimd.partition_broadcast(rbs, rinv)
                            nc.vector.tensor_mul(
                                ot_sb[:, h, it * 512:(it + 1) * 512],
                                otp2[:, it % 2, :], rbs
                            )

                        def _stj(it, jt):
                            return dict(
                                start=(jt == 0),
                                stop=(jt == ((4 * it + 3) if causal
                                             else NJT - 1)),
                            )

                        issue_logits(0)
                        for ui, (kind, j0) in enumerate(units):
                            lp2 = lp_live.pop(ui)
                            e2 = ep.tile([P, 2, 512], bf16, tag="e")
                            if kind == "h":
                                its = its_of(j0)
                                qlos = [qlo_of(it, j0) for it in its]
                                # exp the union range of both halves in one
                                # ACT instr (narrowed MMs read only their
                                # own ranges).
                                qmin = min(qlos)
                                ii0 = its[0] % 2
                                nit = len(its)
                                nc.scalar.activation(
                                    e2[:, ii0:ii0 + nit, qmin:512],
                                    lp2[:, ii0:ii0 + nit, qmin:512],
                                    mybir.ActivationFunctionType.Exp,
                                    scale=SCALE,
                                )
                                halves = [(it, it % 2, j0, q0)
                                          for it, q0 in zip(its, qlos)]
                            else:
                                it = pits[1]
                                qlos = [qlo_of(it, j0 + k) for k in range(2)]
                                qmin = min(qlos)
                                nc.scalar.activation(
                                    e2[:, 0:2, qmin:512],
                                    lp2[:, 0:2, qmin:512],
                                    mybir.ActivationFunctionType.Exp,
                                    scale=SCALE,
                                )
                                halves = [(it, k, j0 + k, qlos[k])
                                          for k in range(2)]
                            for it, hx, jt, q0 in halves:
                                if causal and it == jt // 4:
                                    # within the narrowed range only the
                                    # first 128 queries are partially
                                    # masked: keep where p <= f - q0.
                                    nc.vector.tensor_mul(
                                        e2[:, hx, q0:q0 + P],
                                        e2[:, hx, q0:q0 + P],
                                        dm_sb[:, 0, 0:P])
                            if ui + 1 < len(units):
                                issue_logits(ui + 1)
                            for it, hx, jt, q0 in halves:
                                nc.tensor.matmul(
                                    otp2[:, it % 2, q0:512],
                                    v_sb[:, jt, h * P:(h + 1) * P],
                                    e2[:, hx, q0:512], **_stj(it, jt)
                                )
                            for it, hx, jt, q0 in halves:
                                nc.tensor.matmul(sp2[0:1, it % 2, q0:512],
                                                 ones_col_r,
                                                 e2[:, hx, q0:512],
                                                 **_stj(it, jt))
                            if causal and kind == "h" and j0 == 4 * pits[0] + 3:
                                _normalize(pits[0])
                        if causal:
                            _normalize(pits[1])
                        else:
                            _normalize(pits[0])
                            _normalize(pits[1])

            # ---------- partial output projection, one mt token chunk ----
            # mt-outer with h inner: O^T weight loads reused across nt.
            def _outproj_mt(b, t, mt):
                    ot_sb = t["ot"]
                    opA = ps.tile([P, 2, 512], f32, tag="ps", name=f"oA{b}_{mt}")
                    opB = ps.tile([P, 2, 512], f32, tag="ps", name=f"oB{b}_{mt}")
                    def op_slice(nt):
                        t = opA if nt < 2 else opB
                        return t[:, nt % 2, :]
                    for h in range(HPC):
                        for nt in range(4):
                            nc.tensor.matmul(
                                op_slice(nt), ot_sb[:, h, mt * P:(mt + 1) * P],
                                wo_sb[:, h, nt * 512:(nt + 1) * 512],
                                start=(h == 0), stop=(h == HPC - 1),
                            )
                    ob4 = obp.tile([P, 2048], bf16, tag="ob")
                    # One whole-tile copy per PSUM tile (2 banks contiguous):
                    # same bytes as four half copies but half the instruction
                    # and WAR-semaphore overhead; engine roles alternate per
                    # mt for ACT/DVE balance.
                    engs = ((nc.scalar.copy, nc.vector.tensor_copy) if mt % 2
                            else (nc.vector.tensor_copy, nc.scalar.copy))
                    engs[0](ob4[:, 0:1024], opA[:, :, :])
                    engs[1](ob4[:, 1024:2048], opB[:, :, :])
                    # out DMA issue on the sync queue (idle during outproj);
                    # ACT is already saturated by its two ob copies per tile.
                    # Two half-tile DMAs so the drain starts after the first
                    # two copies instead of all four.
                    for half in range(2):
                        nc.sync.dma_start(
                            out=out.ap()[b * S + mt * P: b * S + (mt + 1) * P,
                                         half * 1024:(half + 1) * 1024],
                            in_=ob4[:, half * 1024:(half + 1) * 1024],
                        )

            if iters > 1:
                with tc.For_i(0, iters, 1):
                    body()
            else:
                body()
    nc.compile()
    return nc


def _get_nc(causal: bool):
    if causal not in _COMPILED:
        _COMPILED[causal] = _build_nc(causal)
    return _COMPILED[causal]


def _numpy_reference(x, mask, Wq, Aq, Bq, Wk, Ak, Bk, Wv, Av, Bv, Wo, Ao, Bo):
    def lora(x2, W, A, Bm):
        return x2 @ W.T + (x2 @ A.T) @ Bm.T

    b, s, d = x.shape
    x2 = x.reshape(b * s, d)

    def heads(t):
        return t.reshape(b, s, H, HD).transpose(0, 2, 1, 3)

    Q = heads(lora(x2, Wq, Aq, Bq).reshape(b, s, d))
    K = heads(lora(x2, Wk, Ak, Bk).reshape(b, s, d))
    V = heads(lora(x2, Wv, Av, Bv).reshape(b, s, d))
    attn = np.einsum("bhqd,bhkd->bhqk", Q, K) / math.sqrt(HD)
    attn = np.where(mask == 0, np.float32(-1e9), attn)
    attn = attn - attn.max(axis=-1, keepdims=True)
    attn = np.exp(attn)
    attn = attn / attn.sum(axis=-1, keepdims=True)
    o = np.einsum("bhqk,bhkd->bhqd", attn, V)
    o = o.transpose(0, 2, 1, 3).reshape(b * s, d)
    return lora(o, Wo, Ao, Bo).reshape(b, s, d).astype(np.float32)


def _make_in_maps(x, wqt, wkt, wvt, wot):
    import ml_dtypes

    bf16 = ml_dtypes.bfloat16
    xt = np.ascontiguousarray(x.reshape(T, D).T.astype(bf16))

    # Diagonal-crossing causal masks: tile (jt=4*it+d, it): keep where
    # 128*d + p_j <= f_i.
    dmn = np.zeros((4, P, 512), dtype=np.float32)
    for dd in range(4):
        pj = np.arange(P)[:, None]
        fi = np.arange(512)[None, :]
        dmn[dd] = (P * dd + pj <= fi).astype(np.float32)
    dmn = dmn.reshape(4 * P, 512).astype(bf16)

    wqt, wkt, wvt, wot = (w.astype(bf16) for w in (wqt, wkt, wvt, wot))
    in_maps = []
    for c in range(NCORES):
        fs = slice(c * FPC, (c + 1) * FPC)
        in_maps.append({
            "xt": xt,
            "wq": np.ascontiguousarray(wqt[:, fs]),
            "wk": np.ascontiguousarray(wkt[:, fs]),
            "wv": np.ascontiguousarray(wvt[:, fs]),
            "wo": np.ascontiguousarray(wot[fs, :]),
            "dm": dmn,
        })
    return in_maps


def kernel(x, mask, Wq, Aq, Bq, Wk, Ak, Bk, Wv, Av, Bv, Wo, Ao, Bo):
    from concourse.bass_utils import run_bass_kernel_spmd

    x = np.asarray(x, dtype=np.float32)
    m2 = np.asarray(mask).reshape(S, S)
    if np.array_equal(m2, np.tril(np.ones((S, S), m2.dtype))):
        causal = True
    elif np.all(m2 != 0):
        causal = False
    else:
        return _numpy_reference(
            np.asarray(x), np.asarray(mask),
            *(np.asarray(a) for a in (Wq, Aq, Bq, Wk, Ak, Bk, Wv, Av, Bv, Wo, Ao, Bo)),
        )

    # Fold LoRA into effective (transposed) weights: W_eff^T = W^T + A^T B^T.
    wqt = (np.asarray(Wq).T + np.asarray(Aq).T @ np.asarray(Bq).T).astype(np.float32)
    wkt = (np.asarray(Wk).T + np.asarray(Ak).T @ np.asarray(Bk).T).astype(np.float32)
    wvt = (np.asarray(Wv).T + np.asarray(Av).T @ np.asarray(Bv).T).astype(np.float32)
    wot = (np.asarray(Wo).T + np.asarray(Ao).T @ np.asarray(Bo).T).astype(np.float32)

    nc = _get_nc(causal)
    in_maps = _make_in_maps(x, wqt, wkt, wvt, wot)
    res = run_bass_kernel_spmd(
        nc, in_maps, list(range(NCORES)),
        trace=bool(int(os.environ.get("KERNEL_TRACE", "0"))),
    )
    if os.environ.get("KERNEL_TRACE") and res.exec_time_ns is not None:
        print(f"HW exec time: {res.exec_time_ns} ns", file=sys.stderr)
        if res.instructions_and_trace is not None:
            print(f"trace path: {res.instructions_and_trace[1]}", file=sys.stderr)
    total = np.zeros((T, D), dtype=np.float32)
    for r in res.results:
        total += np.asarray(r["out"]).astype(np.float32)
    return total.reshape(B, S, D)

